# revision 37
# baseline (speedup 1.0000x reference)
"""Trainium2 Bass kernel for nn_Attention_56470230008033.

Multi-head self-attention (B=2, N=2048, C=1024, H=16 heads, D=64),
k = v = q, full qkv projection + output projection.

Sharding over 8 NeuronCores: data parallel on batch (2) x tensor
parallel on heads (4 head-groups of 4 heads).

Fused streaming design (v2): the scalar engine's exp stream over the
4x2048x2048 attention matrix (~110us of ACT work) is the critical
path, so everything else is scheduled around keeping it dense:
  - x and weights are passed from host in bf16; x is DMA'd with the
    XBAR transpose directly DRAM -> SBUF (no staging, no PE transpose)
  - logits^T per head pair via row-tiled concurrent K=64 matmuls
    (heads at PE rows 0-63 / 64-127, separate PSUM banks)
  - softmax denominators via the ones-column-in-V trick (65-col PV)
  - output projection with K=128 (two heads packed per contraction)
  - per-chunk (512 query rows) pipeline: B -> exp -> PV, with the
    QKV projections for later pairs interleaved into PE slack, and
    y projection + DMA-out streamed per chunk of the second pair
"""

import os
import sys

for _p in ("/opt/trn_rl_repo", "/opt/pypackages"):
    if _p not in sys.path:
        sys.path.append(_p)

import numpy as np

_DEBUG = os.environ.get("KDEBUG") == "1"

B, N, C, H = 2, 2048, 1024, 16
D = C // H            # 64 head dim
NCORES = 8
HPC = 4               # heads per core
F = HPC * D           # 256 features per core
NT = N // 128         # 16 token tiles
CT = C // 128         # 8 contraction tiles
NCH = N // 512        # 4 chunks of 512

_CACHE = {}


def _build():
    from concourse import bacc, bass, mybir, tile, masks

    F32 = mybir.dt.float32
    BF16 = mybir.dt.bfloat16
    AF = mybir.ActivationFunctionType

    nc = bacc.Bacc(
        "TRN2",
        target_bir_lowering=False,
        debug=False,
        enable_asserts=False,
        num_devices=NCORES,
    )
    x_d = nc.dram_tensor("x", [N, C], BF16, kind="ExternalInput")
    # weights arrive partition-packed from the host so each loads in one
    # contiguous near-line-rate DMA. wqk row p = [Q01K01 c=0..7 | Q23K23
    # c=0..7], 256 cols per c-tile ([Q|K] 128 each).
    wqk_d = nc.dram_tensor("wqk", [128, 2 * CT * 256], BF16, kind="ExternalInput")
    wv_d = nc.dram_tensor("wv", [128, CT * F], BF16, kind="ExternalInput")
    wp_d = nc.dram_tensor("wp", [128, 2 * C], BF16, kind="ExternalInput")
    bqk_d = nc.dram_tensor("bqk", [128, 4], F32, kind="ExternalInput")
    bv_d = nc.dram_tensor("bv", [1, F], F32, kind="ExternalInput")
    y_d = nc.dram_tensor("y", [N, C], F32, kind="ExternalOutput")
    if _DEBUG:
        qk_dump = nc.dram_tensor("qk_dump", [4 * 128, N], BF16, kind="ExternalOutput")
        va_dump = nc.dram_tensor("va_dump", [NT * 128, HPC * (D + 1)], BF16,
                                 kind="ExternalOutput")
        ot_dump = nc.dram_tensor("ot_dump", [2 * 128, N], BF16, kind="ExternalOutput")

    with tile.TileContext(nc) as tc:
        from contextlib import ExitStack

        with ExitStack() as ctx:
            const = ctx.enter_context(tc.tile_pool(name="const", bufs=1))
            persist = ctx.enter_context(tc.tile_pool(name="persist", bufs=1))
            ptp = ctx.enter_context(tc.tile_pool(name="ptp", bufs=8))
            ysb = ctx.enter_context(tc.tile_pool(name="ysb", bufs=2))
            snorm = ctx.enter_context(tc.tile_pool(name="snorm", bufs=2))

            ident = const.tile([128, 128], BF16, name="ident", tag="ident")
            masks.make_identity(nc, ident[:])
            junk = const.tile([128, 512], BF16, name="junk", tag="junk")
            nc.vector.memset(junk[:], 0.0)
            ones64 = const.tile([1, D], F32, name="ones64", tag="ones64")
            nc.vector.memset(ones64[:], 1.0)

            # persistent SBUF tensors (all bf16 from host)
            xTall = persist.tile([128, CT * N], BF16, name="xTall", tag="xTall")
            xT3 = xTall.rearrange("p (c n) -> p c n", c=CT)
            wqk3 = persist.tile([128, 2, CT, 256], BF16, name="wqk3", tag="wqk3")
            wv3 = persist.tile([128, CT, F], BF16, name="wv3", tag="wv3")
            wpp3 = persist.tile([128, 2, C], BF16, name="wpp3", tag="wpp3")
            # qkT[0]=Q01 qkT[1]=K01 qkT[2]=Q23 qkT[3]=K23; per pair the
            # even head sits at rows 0-63, odd head at rows 64-127
            qkT = [persist.tile([128, N], BF16, name=f"qkT{f}", tag=f"qkT{f}")
                   for f in range(4)]
            # V with ones column per head: [128, h, 65]
            vaug = [persist.tile([128, HPC, D + 1], BF16, name=f"vaug{t}", tag=f"vaug{t}")
                    for t in range(NT)]
            oTp = [persist.tile([128, N], BF16, name=f"oTp{hp}", tag=f"oTp{hp}")
                   for hp in range(2)]
            bqk_sb = const.tile([128, 4], F32, name="bqk_sb", tag="bqk_sb")
            bvb = const.tile([128, F], F32, name="bvb", tag="bvb")
            bvb3 = bvb.rearrange("p (h d) -> p h d", h=HPC)

            # ---- front-loaded DMA issue, all on the sync (HWDGE) ring.
            # DMA_TRANSPOSE serializes against other DMA traffic, so the
            # weight loads are interleaved between the 4 x-transposes in
            # consumption order rather than on a second ring.
            def xpose(g):
                nc.sync.dma_start(xT3[:, :, g * 512:(g + 1) * 512],
                                  x_d.ap()[g * 512:(g + 1) * 512, :],
                                  transpose=True)

            wqk_dv = wqk_d.ap().rearrange("p (w c f) -> p w c f", w=2, c=CT)
            nc.sync.dma_start(wqk3[:, 0], wqk_dv[:, 0])      # Q01|K01
            nc.sync.dma_start(bqk_sb[:], bqk_d.ap())
            nc.sync.dma_start(wv3[:], wv_d.ap().rearrange("p (c f) -> p c f", c=CT))
            bv1 = const.tile([1, F], F32, name="bv1", tag="bv1")
            nc.sync.dma_start(bv1[:], bv_d.ap())
            nc.gpsimd.partition_broadcast(bvb[:], bv1[:])
            # the DIRECT2D->TRANSPOSE transition pays a ~3us serialization
            # gap, so issue all four transposes back-to-back after the
            # pair-0-critical weights
            xpose(0)
            xpose(1)
            xpose(2)
            xpose(3)
            # pair-1 weights are not needed until ~mid-stream; keep them
            # behind the transposes so chunk 0's K tiles arrive sooner
            nc.sync.dma_start(wqk3[:, 1], wqk_dv[:, 1])      # Q23|K23
            nc.sync.dma_start(wpp3[:], wp_d.ap().rearrange("p (a c) -> p a c", a=2))

            # exp table preload on the scalar engine (one-time ~2.7us)
            scr = const.tile([1, 16], F32, name="scr", tag="scr")
            nc.scalar.activation(scr[:], ident[0:1, 0:16], AF.Exp)

            # ones columns of vaug (never overwritten afterwards)
            for t in range(NT):
                nc.vector.memset(vaug[t][:, :, D:D + 1], 1.0)

            with tc.tile_pool(name="bpp", bufs=2, space=bass.MemorySpace.PSUM) as bpp, \
                 tc.tile_pool(name="cpp", bufs=1, space=bass.MemorySpace.PSUM) as cpp:

                def qk_proj(f, j):
                    # project qkT[f] n-cols j*512:(j+1)*512
                    w, fh = divmod(f, 2)
                    qp = qpp.tile([128, 512], F32, name="qp", tag="qp")
                    for c in range(CT):
                        nc.tensor.matmul(
                            qp[:], wqk3[:, w, c, fh * 128:(fh + 1) * 128],
                            xT3[:, c, j * 512:(j + 1) * 512],
                            start=(c == 0), stop=(c == CT - 1))
                    nc.vector.tensor_scalar_add(
                        qkT[f][:, j * 512:(j + 1) * 512], qp[:], bqk_sb[:, f:f + 1])

                def v_proj(t):
                    vp = vpp.tile([128, F], F32, name="vp", tag="vp")
                    for c in range(CT):
                        nc.tensor.matmul(
                            vp[:], xT3[:, c, t * 128:(t + 1) * 128], wv3[:, c],
                            start=(c == 0), stop=(c == CT - 1))
                    nc.vector.tensor_add(
                        vaug[t][:, :, 0:D],
                        vp.rearrange("p (h d) -> p h d", h=HPC), bvb3)

                def y_sub(c, s):
                    # per-512-col halves so the psum copy of one half
                    # overlaps the projection of the next
                    t = c * 4 + s
                    ys = ysb.tile([128, C], F32, name="ys", tag="ys")
                    for half in range(2):
                        yp = ypp.tile([128, 512], F32, name="yp", tag="yp")
                        for hp in range(2):
                            nc.tensor.matmul(
                                yp[:],
                                oTp[hp][:, t * 128:(t + 1) * 128],
                                wpp3[:, hp, half * 512:(half + 1) * 512],
                                start=(hp == 0), stop=(hp == 1))
                        nc.vector.tensor_copy(ys[:, half * 512:(half + 1) * 512], yp[:])
                    nc.sync.dma_start(y_d.ap()[t * 128:(t + 1) * 128, :], ys[:])

                def run_chunk(p, c, hooks, pre=()):
                    qt, kt = qkT[2 * p], qkT[2 * p + 1]
                    nb = c * 512
                    cp = cpp.tile([D + 1, 2, 512], F32, name="cp", tag="cp")

                    def bmm(mt):
                        bp = bpp.tile([128, 2, 512], F32, name="bp", tag="bp")
                        nc.tensor.matmul(
                            bp[:, 0], kt[0:D, mt * 128:(mt + 1) * 128],
                            qt[0:D, nb:nb + 512], start=True, stop=True)
                        nc.tensor.matmul(
                            bp[:, 1], kt[D:2 * D, mt * 128:(mt + 1) * 128],
                            qt[D:2 * D, nb:nb + 512], start=True, stop=True)
                        return bp

                    # pre-hooks must precede the bmm look-ahead init: chunk
                    # 0's bmm(4..5) consume the K01 m-chunk projected in pre
                    for hook in pre:
                        hook()
                    bps = {mt: bmm(mt) for mt in range(6)}
                    for mt in range(NT):
                        pt = ptp.tile([128, 2, 512], BF16, name="pt", tag="pt")
                        nc.scalar.activation(
                            pt.rearrange("p a b -> p (a b)"),
                            bps.pop(mt).rearrange("p a b -> p (a b)"),
                            AF.Exp, scale=float(D) ** -0.5)
                        nc.tensor.matmul(cp[:, 0], vaug[mt][:, 2 * p, :], pt[:, 0],
                                         start=(mt == 0), stop=(mt == NT - 1))
                        nc.tensor.matmul(cp[:, 1], vaug[mt][:, 2 * p + 1, :], pt[:, 1],
                                         start=(mt == 0), stop=(mt == NT - 1))
                        for hook in hooks.get(mt, ()):
                            hook()
                        if mt + 6 < NT:
                            bps[mt + 6] = bmm(mt + 6)
                    # normalize: oTp rows = cp[0:D] * (1/cp[D]) per head,
                    # both heads batched per DVE op. DVE lanes are
                    # partition-locked: the den row is first copied to
                    # partition 0 (tensor_copy handles the cross-partition
                    # move; reciprocal would not), and the odd head goes
                    # through a partition-0 scratch + SBUF->SBUF DMA to
                    # reach partitions 64-127.
                    s0 = snorm.tile([1, 2, 512], F32, name="s0", tag="s0")
                    nc.vector.tensor_copy(s0[:], cp[D:D + 1, :])
                    sr = snorm.tile([1, 2 * 512], F32, name="sr", tag="sr")
                    nc.vector.reciprocal_approx_fast(
                        sr[:], s0.rearrange("p a b -> p (a b)"))
                    sb = snorm.tile([D, 2, 512], F32, name="sb", tag="sb")
                    nc.gpsimd.partition_broadcast(
                        sb.rearrange("p a b -> p (a b)"), sr[:])
                    nc.vector.tensor_mul(
                        oTp[p][0:D, nb:nb + 512], cp[0:D, 0], sb[:, 0])
                    om = snorm.tile([D, 512], BF16, name="om", tag="om")
                    nc.vector.tensor_mul(om[:], cp[0:D, 1], sb[:, 1])
                    nc.sync.dma_start(oTp[p][D:2 * D, nb:nb + 512], om[:])

                # ---- pair 0: projections + chunk stream ----
                with tc.tile_pool(name="qpp", bufs=1, space=bass.MemorySpace.PSUM) as qpp, \
                     tc.tile_pool(name="vpp", bufs=1, space=bass.MemorySpace.PSUM) as vpp:
                    # HAM warm-up fillers (junk, never read): enough to keep
                    # the PE busy until the first x-transpose lands, so the
                    # Q/K projections run at the warm 2.4 GHz clock
                    for _ in range(28):
                        wt = qpp.tile([128, 512], F32, name="wt", tag="qp")
                        nc.tensor.matmul(wt[:], ident[:], junk[:],
                                         start=True, stop=True)
                    qk_proj(0, 0)      # Q01 chunk 0
                    qk_proj(1, 0)      # K01 m-chunk 0

                    # K01 m-chunk k must be emitted before bmm(4k) — with the
                    # 4-deep bmm look-ahead that means hook slot 4k-4 at the
                    # latest (hooks run before the look-ahead bmm).
                    hooks0 = {mt: [(lambda t=mt + 2: v_proj(t))]
                              for mt in range(NT - 2)}
                    for mk, mt in ((2, 1), (3, 5)):
                        hooks0[mt].append(lambda mk=mk: qk_proj(1, mk))
                    hooks0[12].append(lambda: qk_proj(0, 1))
                    run_chunk(0, 0, hooks0,
                              pre=(lambda: v_proj(0), lambda: v_proj(1),
                                   lambda: qk_proj(1, 1)))
                    run_chunk(0, 1, {2: [lambda: qk_proj(3, 0)],
                                     6: [lambda: qk_proj(3, 1)],
                                     10: [lambda: qk_proj(0, 2)]})
                    run_chunk(0, 2, {2: [lambda: qk_proj(3, 2)],
                                     6: [lambda: qk_proj(3, 3)],
                                     10: [lambda: qk_proj(0, 3)]})
                    run_chunk(0, 3, {2: [lambda: qk_proj(2, 0)],
                                     5: [lambda: qk_proj(2, 1)],
                                     8: [lambda: qk_proj(2, 2)],
                                     11: [lambda: qk_proj(2, 3)]})

                # ---- pair 1: chunk stream + streamed y projection ----
                with tc.tile_pool(name="ypp", bufs=2, space=bass.MemorySpace.PSUM) as ypp:
                    run_chunk(1, 0, {})
                    run_chunk(1, 1, {mt: [(lambda s=s: y_sub(0, s))]
                                     for s, mt in enumerate((2, 5, 8, 11))})
                    run_chunk(1, 2, {mt: [(lambda s=s: y_sub(1, s))]
                                     for s, mt in enumerate((2, 5, 8, 11))})
                    run_chunk(1, 3, {mt: [(lambda s=s: y_sub(2, s))]
                                     for s, mt in enumerate((2, 5, 8, 11))})
                    for s in range(4):
                        y_sub(3, s)

                if _DEBUG:
                    for f in range(4):
                        nc.sync.dma_start(qk_dump.ap()[f * 128:(f + 1) * 128, :],
                                          qkT[f][:])
                    for t in range(NT):
                        nc.sync.dma_start(va_dump.ap()[t * 128:(t + 1) * 128, :],
                                          vaug[t].rearrange("p h d -> p (h d)"))
                    for hp in range(2):
                        nc.sync.dma_start(ot_dump.ap()[hp * 128:(hp + 1) * 128, :],
                                          oTp[hp][:])

    nc.compile()
    return nc


def _get_nc():
    if "nc" not in _CACHE:
        _CACHE["nc"] = _build()
    return _CACHE["nc"]


def _in_maps(q, W_qkv, b_qkv, W_proj):
    import ml_dtypes

    bf16 = ml_dtypes.bfloat16
    maps = []
    Wq, Wk, Wv = W_qkv[:, :C], W_qkv[:, C:2 * C], W_qkv[:, 2 * C:]
    bq, bk, bv = b_qkv[:C], b_qkv[C:2 * C], b_qkv[2 * C:]
    def pack(w):
        # [1024, X] -> partition-packed [128, 8*X]: row p = [c=0..7 slices]
        return w.reshape(CT, 128, -1).transpose(1, 0, 2).reshape(128, -1)

    for core in range(NCORES):
        b, g = divmod(core, HPC)
        cols = slice(g * F, (g + 1) * F)
        wqg, wkg = Wq[:, cols], Wk[:, cols]
        # per c-tile cols: [Q01|K01] then [Q23|K23]
        w0 = pack(np.concatenate([wqg[:, 0:128], wkg[:, 0:128]], axis=1))
        w1 = pack(np.concatenate([wqg[:, 128:256], wkg[:, 128:256]], axis=1))
        maps.append({
            "x": np.ascontiguousarray(q[b].astype(bf16)),
            "wqk": np.ascontiguousarray(
                np.concatenate([w0, w1], axis=1).astype(bf16)),
            "wv": np.ascontiguousarray(pack(Wv[:, cols]).astype(bf16)),
            "wp": np.ascontiguousarray(
                W_proj[cols, :].reshape(2, 128, C)
                .transpose(1, 0, 2).reshape(128, 2 * C).astype(bf16)),
            "bqk": np.ascontiguousarray(np.stack(
                [bq[cols][0:128], bk[cols][0:128],
                 bq[cols][128:256], bk[cols][128:256]], axis=1)),
            "bv": np.ascontiguousarray(bv[cols].reshape(1, F)),
        })
    return maps


def kernel(q, W_qkv, b_qkv, W_proj, b_proj):
    from concourse.bass_utils import run_bass_kernel_spmd

    q = np.ascontiguousarray(np.asarray(q, dtype=np.float32))
    W_qkv = np.ascontiguousarray(np.asarray(W_qkv, dtype=np.float32))
    b_qkv = np.ascontiguousarray(np.asarray(b_qkv, dtype=np.float32))
    W_proj = np.ascontiguousarray(np.asarray(W_proj, dtype=np.float32))
    b_proj = np.ascontiguousarray(np.asarray(b_proj, dtype=np.float32))

    nc = _get_nc()
    res = run_bass_kernel_spmd(nc, _in_maps(q, W_qkv, b_qkv, W_proj),
                               core_ids=list(range(NCORES)))

    out = np.zeros((B, N, C), dtype=np.float32)
    for core in range(NCORES):
        out[core // HPC] += res.results[core]["y"]
    out += b_proj
    return out


# revision 38
# speedup vs baseline: 1.0443x; 1.0443x over previous
"""Trainium2 Bass kernel for nn_Attention_56470230008033.

Multi-head self-attention (B=2, N=2048, C=1024, H=16 heads, D=64),
k = v = q, full qkv projection + output projection.

Sharding over 8 NeuronCores: data parallel on batch (2) x tensor
parallel on heads (4 head-groups of 4 heads).

Fused streaming design (v2): the scalar engine's exp stream over the
4x2048x2048 attention matrix (~110us of ACT work) is the critical
path, so everything else is scheduled around keeping it dense:
  - x and weights are passed from host in bf16; x is DMA'd with the
    XBAR transpose directly DRAM -> SBUF (no staging, no PE transpose)
  - logits^T per head pair via row-tiled concurrent K=64 matmuls
    (heads at PE rows 0-63 / 64-127, separate PSUM banks)
  - softmax denominators via the ones-column-in-V trick (65-col PV)
  - output projection with K=128 (two heads packed per contraction)
  - per-chunk (512 query rows) pipeline: B -> exp -> PV, with the
    QKV projections for later pairs interleaved into PE slack, and
    y projection + DMA-out streamed per chunk of the second pair
"""

import os
import sys

for _p in ("/opt/trn_rl_repo", "/opt/pypackages"):
    if _p not in sys.path:
        sys.path.append(_p)

import numpy as np

_DEBUG = os.environ.get("KDEBUG") == "1"

B, N, C, H = 2, 2048, 1024, 16
D = C // H            # 64 head dim
NCORES = 8
HPC = 4               # heads per core
F = HPC * D           # 256 features per core
NT = N // 128         # 16 token tiles
CT = C // 128         # 8 contraction tiles
NCH = N // 512        # 4 chunks of 512

_CACHE = {}


def _build():
    from concourse import bacc, bass, mybir, tile, masks

    F32 = mybir.dt.float32
    BF16 = mybir.dt.bfloat16
    AF = mybir.ActivationFunctionType

    nc = bacc.Bacc(
        "TRN2",
        target_bir_lowering=False,
        debug=False,
        enable_asserts=False,
        num_devices=NCORES,
    )
    x_d = nc.dram_tensor("x", [N, C], BF16, kind="ExternalInput")
    # weights arrive partition-packed from the host so each loads in one
    # contiguous near-line-rate DMA. wqk row p = [Q01K01 c=0..7 | Q23K23
    # c=0..7], 256 cols per c-tile ([Q|K] 128 each).
    wqk_d = nc.dram_tensor("wqk", [128, 2 * CT * 256], BF16, kind="ExternalInput")
    wv_d = nc.dram_tensor("wv", [128, CT * F], BF16, kind="ExternalInput")
    wp_d = nc.dram_tensor("wp", [128, 2 * C], BF16, kind="ExternalInput")
    bqk_d = nc.dram_tensor("bqk", [128, 4], F32, kind="ExternalInput")
    bv_d = nc.dram_tensor("bv", [1, F], F32, kind="ExternalInput")
    y_d = nc.dram_tensor("y", [N, C], F32, kind="ExternalOutput")
    if _DEBUG:
        qk_dump = nc.dram_tensor("qk_dump", [4 * 128, N], BF16, kind="ExternalOutput")
        va_dump = nc.dram_tensor("va_dump", [NT * 128, HPC * (D + 1)], BF16,
                                 kind="ExternalOutput")
        ot_dump = nc.dram_tensor("ot_dump", [2 * 128, N], BF16, kind="ExternalOutput")

    with tile.TileContext(nc) as tc:
        from contextlib import ExitStack

        with ExitStack() as ctx:
            const = ctx.enter_context(tc.tile_pool(name="const", bufs=1))
            persist = ctx.enter_context(tc.tile_pool(name="persist", bufs=1))
            ptp = ctx.enter_context(tc.tile_pool(name="ptp", bufs=8))
            ysb = ctx.enter_context(tc.tile_pool(name="ysb", bufs=2))
            snorm = ctx.enter_context(tc.tile_pool(name="snorm", bufs=2))

            ident = const.tile([128, 128], BF16, name="ident", tag="ident")
            masks.make_identity(nc, ident[:])
            junk = const.tile([128, 512], BF16, name="junk", tag="junk")
            nc.vector.memset(junk[:], 0.0)
            ones64 = const.tile([1, D], F32, name="ones64", tag="ones64")
            nc.vector.memset(ones64[:], 1.0)

            # persistent SBUF tensors (all bf16 from host)
            xTall = persist.tile([128, CT * N], BF16, name="xTall", tag="xTall")
            xT3 = xTall.rearrange("p (c n) -> p c n", c=CT)
            wqk3 = persist.tile([128, 2, CT, 256], BF16, name="wqk3", tag="wqk3")
            wv3 = persist.tile([128, CT, F], BF16, name="wv3", tag="wv3")
            wpp3 = persist.tile([128, 2, C], BF16, name="wpp3", tag="wpp3")
            # qkT[0]=Q01 qkT[1]=K01 qkT[2]=Q23 qkT[3]=K23; per pair the
            # even head sits at rows 0-63, odd head at rows 64-127
            qkT = [persist.tile([128, N], BF16, name=f"qkT{f}", tag=f"qkT{f}")
                   for f in range(4)]
            # V with ones column per head: [128, h, 65]
            vaug = [persist.tile([128, HPC, D + 1], BF16, name=f"vaug{t}", tag=f"vaug{t}")
                    for t in range(NT)]
            oTp = [persist.tile([128, N], BF16, name=f"oTp{hp}", tag=f"oTp{hp}")
                   for hp in range(2)]
            bqk_sb = const.tile([128, 4], F32, name="bqk_sb", tag="bqk_sb")
            bvb = const.tile([128, F], F32, name="bvb", tag="bvb")
            bvb3 = bvb.rearrange("p (h d) -> p h d", h=HPC)

            # ---- front-loaded DMA issue, all on the sync (HWDGE) ring.
            # DMA_TRANSPOSE serializes against other DMA traffic, so the
            # weight loads are interleaved between the 4 x-transposes in
            # consumption order rather than on a second ring.
            def xpose(g):
                nc.sync.dma_start(xT3[:, :, g * 512:(g + 1) * 512],
                                  x_d.ap()[g * 512:(g + 1) * 512, :],
                                  transpose=True)

            wqk_dv = wqk_d.ap().rearrange("p (w c f) -> p w c f", w=2, c=CT)
            nc.sync.dma_start(wqk3[:, 0], wqk_dv[:, 0])      # Q01|K01
            nc.sync.dma_start(bqk_sb[:], bqk_d.ap())
            nc.sync.dma_start(wv3[:], wv_d.ap().rearrange("p (c f) -> p c f", c=CT))
            bv1 = const.tile([1, F], F32, name="bv1", tag="bv1")
            nc.sync.dma_start(bv1[:], bv_d.ap())
            nc.gpsimd.partition_broadcast(bvb[:], bv1[:])
            # the DIRECT2D->TRANSPOSE transition pays a ~3us serialization
            # gap, so issue all four transposes back-to-back after the
            # pair-0-critical weights
            xpose(0)
            xpose(1)
            xpose(2)
            xpose(3)
            # pair-1 weights are not needed until ~mid-stream; keep them
            # behind the transposes so chunk 0's K tiles arrive sooner
            nc.sync.dma_start(wqk3[:, 1], wqk_dv[:, 1])      # Q23|K23
            nc.sync.dma_start(wpp3[:], wp_d.ap().rearrange("p (a c) -> p a c", a=2))

            # exp table preload on the scalar engine (one-time ~2.7us)
            scr = const.tile([1, 16], F32, name="scr", tag="scr")
            nc.scalar.activation(scr[:], ident[0:1, 0:16], AF.Exp)

            # ones columns of vaug (never overwritten afterwards)
            for t in range(NT):
                nc.vector.memset(vaug[t][:, :, D:D + 1], 1.0)

            with tc.tile_pool(name="bpp", bufs=2, space=bass.MemorySpace.PSUM) as bpp, \
                 tc.tile_pool(name="cpp", bufs=1, space=bass.MemorySpace.PSUM) as cpp:

                def qk_proj(f, j):
                    # project qkT[f] n-cols j*512:(j+1)*512
                    w, fh = divmod(f, 2)
                    qp = qpp.tile([128, 512], F32, name="qp", tag="qp")
                    for c in range(CT):
                        nc.tensor.matmul(
                            qp[:], wqk3[:, w, c, fh * 128:(fh + 1) * 128],
                            xT3[:, c, j * 512:(j + 1) * 512],
                            start=(c == 0), stop=(c == CT - 1))
                    nc.vector.tensor_scalar_add(
                        qkT[f][:, j * 512:(j + 1) * 512], qp[:], bqk_sb[:, f:f + 1])

                def v_proj(t):
                    vp = vpp.tile([128, F], F32, name="vp", tag="vp")
                    for c in range(CT):
                        nc.tensor.matmul(
                            vp[:], xT3[:, c, t * 128:(t + 1) * 128], wv3[:, c],
                            start=(c == 0), stop=(c == CT - 1))
                    nc.vector.tensor_add(
                        vaug[t][:, :, 0:D],
                        vp.rearrange("p (h d) -> p h d", h=HPC), bvb3)

                def y_sub(c, s):
                    # per-512-col halves so the psum copy of one half
                    # overlaps the projection of the next
                    t = c * 4 + s
                    ys = ysb.tile([128, C], F32, name="ys", tag="ys")
                    for half in range(2):
                        yp = ypp.tile([128, 512], F32, name="yp", tag="yp")
                        for hp in range(2):
                            nc.tensor.matmul(
                                yp[:],
                                oTp[hp][:, t * 128:(t + 1) * 128],
                                wpp3[:, hp, half * 512:(half + 1) * 512],
                                start=(hp == 0), stop=(hp == 1))
                        nc.vector.tensor_copy(ys[:, half * 512:(half + 1) * 512], yp[:])
                    nc.sync.dma_start(y_d.ap()[t * 128:(t + 1) * 128, :], ys[:])

                def run_chunk(p, c, hooks, pre=()):
                    qt, kt = qkT[2 * p], qkT[2 * p + 1]
                    nb = c * 512
                    cp = cpp.tile([D + 1, 2, 512], F32, name="cp", tag="cp")

                    def bmm(mt):
                        bp = bpp.tile([128, 2, 512], F32, name="bp", tag="bp")
                        nc.tensor.matmul(
                            bp[:, 0], kt[0:D, mt * 128:(mt + 1) * 128],
                            qt[0:D, nb:nb + 512], start=True, stop=True)
                        nc.tensor.matmul(
                            bp[:, 1], kt[D:2 * D, mt * 128:(mt + 1) * 128],
                            qt[D:2 * D, nb:nb + 512], start=True, stop=True)
                        return bp

                    # pre-hooks must precede the bmm look-ahead init: chunk
                    # 0's bmm(4..5) consume the K01 m-chunk projected in pre
                    for hook in pre:
                        hook()
                    bps = {mt: bmm(mt) for mt in range(6)}
                    for mt in range(NT):
                        pt = ptp.tile([128, 2, 512], BF16, name="pt", tag="pt")
                        nc.scalar.activation(
                            pt.rearrange("p a b -> p (a b)"),
                            bps.pop(mt).rearrange("p a b -> p (a b)"),
                            AF.Exp, scale=float(D) ** -0.5)
                        nc.tensor.matmul(cp[:, 0], vaug[mt][:, 2 * p, :], pt[:, 0],
                                         start=(mt == 0), stop=(mt == NT - 1))
                        nc.tensor.matmul(cp[:, 1], vaug[mt][:, 2 * p + 1, :], pt[:, 1],
                                         start=(mt == 0), stop=(mt == NT - 1))
                        for hook in hooks.get(mt, ()):
                            hook()
                        if mt + 6 < NT:
                            bps[mt + 6] = bmm(mt + 6)
                    # normalize: oTp rows = cp[0:D] * (1/cp[D]) per head,
                    # both heads batched per DVE op. DVE lanes are
                    # partition-locked: the den row is first copied to
                    # partition 0 (tensor_copy handles the cross-partition
                    # move; reciprocal would not), and the odd head goes
                    # through a partition-0 scratch + SBUF->SBUF DMA to
                    # reach partitions 64-127.
                    s0 = snorm.tile([1, 2, 512], F32, name="s0", tag="s0")
                    nc.vector.tensor_copy(s0[:], cp[D:D + 1, :])
                    # stage the value rows to SBUF right away so the cp
                    # psum bank frees before the recip/broadcast chain —
                    # the next chunk's first PV no longer waits on it
                    cbig = snorm.tile([D, 2, 512], F32, name="cbig", tag="cbig")
                    nc.vector.tensor_copy(cbig[:], cp[0:D, :])
                    sr = snorm.tile([1, 2 * 512], F32, name="sr", tag="sr")
                    nc.vector.reciprocal_approx_fast(
                        sr[:], s0.rearrange("p a b -> p (a b)"))
                    sb = snorm.tile([D, 2, 512], F32, name="sb", tag="sb")
                    nc.gpsimd.partition_broadcast(
                        sb.rearrange("p a b -> p (a b)"), sr[:])
                    nc.vector.tensor_mul(
                        oTp[p][0:D, nb:nb + 512], cbig[:, 0], sb[:, 0])
                    om = snorm.tile([D, 512], BF16, name="om", tag="om")
                    nc.vector.tensor_mul(om[:], cbig[:, 1], sb[:, 1])
                    nc.sync.dma_start(oTp[p][D:2 * D, nb:nb + 512], om[:])

                # ---- pair 0: projections + chunk stream ----
                with tc.tile_pool(name="qpp", bufs=1, space=bass.MemorySpace.PSUM) as qpp, \
                     tc.tile_pool(name="vpp", bufs=1, space=bass.MemorySpace.PSUM) as vpp:
                    # HAM warm-up fillers (junk, never read): enough to keep
                    # the PE busy until the first x-transpose lands, so the
                    # Q/K projections run at the warm 2.4 GHz clock
                    for _ in range(28):
                        wt = qpp.tile([128, 512], F32, name="wt", tag="qp")
                        nc.tensor.matmul(wt[:], ident[:], junk[:],
                                         start=True, stop=True)
                    qk_proj(0, 0)      # Q01 chunk 0
                    qk_proj(1, 0)      # K01 m-chunk 0

                    # K01 m-chunk k must be emitted before bmm(4k) — with the
                    # 4-deep bmm look-ahead that means hook slot 4k-4 at the
                    # latest (hooks run before the look-ahead bmm).
                    hooks0 = {mt: [(lambda t=mt + 2: v_proj(t))]
                              for mt in range(NT - 2)}
                    for mk, mt in ((2, 1), (3, 5)):
                        hooks0[mt].append(lambda mk=mk: qk_proj(1, mk))
                    hooks0[12].append(lambda: qk_proj(0, 1))
                    run_chunk(0, 0, hooks0,
                              pre=(lambda: v_proj(0), lambda: v_proj(1),
                                   lambda: qk_proj(1, 1)))
                    run_chunk(0, 1, {2: [lambda: qk_proj(3, 0)],
                                     6: [lambda: qk_proj(3, 1)],
                                     10: [lambda: qk_proj(0, 2)]})
                    run_chunk(0, 2, {2: [lambda: qk_proj(3, 2)],
                                     6: [lambda: qk_proj(3, 3)],
                                     10: [lambda: qk_proj(0, 3)]})
                    run_chunk(0, 3, {2: [lambda: qk_proj(2, 0)],
                                     5: [lambda: qk_proj(2, 1)],
                                     8: [lambda: qk_proj(2, 2)],
                                     11: [lambda: qk_proj(2, 3)]})

                # ---- pair 1: chunk stream + streamed y projection ----
                with tc.tile_pool(name="ypp", bufs=2, space=bass.MemorySpace.PSUM) as ypp:
                    run_chunk(1, 0, {})
                    run_chunk(1, 1, {mt: [(lambda s=s: y_sub(0, s))]
                                     for s, mt in enumerate((2, 5, 8, 11))})
                    run_chunk(1, 2, {mt: [(lambda s=s: y_sub(1, s))]
                                     for s, mt in enumerate((2, 5, 8, 11))})
                    run_chunk(1, 3, {mt: [(lambda s=s: y_sub(2, s))]
                                     for s, mt in enumerate((2, 5, 8, 11))})
                    for s in range(4):
                        y_sub(3, s)

                if _DEBUG:
                    for f in range(4):
                        nc.sync.dma_start(qk_dump.ap()[f * 128:(f + 1) * 128, :],
                                          qkT[f][:])
                    for t in range(NT):
                        nc.sync.dma_start(va_dump.ap()[t * 128:(t + 1) * 128, :],
                                          vaug[t].rearrange("p h d -> p (h d)"))
                    for hp in range(2):
                        nc.sync.dma_start(ot_dump.ap()[hp * 128:(hp + 1) * 128, :],
                                          oTp[hp][:])

    nc.compile()
    return nc


def _get_nc():
    if "nc" not in _CACHE:
        _CACHE["nc"] = _build()
    return _CACHE["nc"]


def _in_maps(q, W_qkv, b_qkv, W_proj):
    import ml_dtypes

    bf16 = ml_dtypes.bfloat16
    maps = []
    Wq, Wk, Wv = W_qkv[:, :C], W_qkv[:, C:2 * C], W_qkv[:, 2 * C:]
    bq, bk, bv = b_qkv[:C], b_qkv[C:2 * C], b_qkv[2 * C:]
    def pack(w):
        # [1024, X] -> partition-packed [128, 8*X]: row p = [c=0..7 slices]
        return w.reshape(CT, 128, -1).transpose(1, 0, 2).reshape(128, -1)

    for core in range(NCORES):
        b, g = divmod(core, HPC)
        cols = slice(g * F, (g + 1) * F)
        wqg, wkg = Wq[:, cols], Wk[:, cols]
        # per c-tile cols: [Q01|K01] then [Q23|K23]
        w0 = pack(np.concatenate([wqg[:, 0:128], wkg[:, 0:128]], axis=1))
        w1 = pack(np.concatenate([wqg[:, 128:256], wkg[:, 128:256]], axis=1))
        maps.append({
            "x": np.ascontiguousarray(q[b].astype(bf16)),
            "wqk": np.ascontiguousarray(
                np.concatenate([w0, w1], axis=1).astype(bf16)),
            "wv": np.ascontiguousarray(pack(Wv[:, cols]).astype(bf16)),
            "wp": np.ascontiguousarray(
                W_proj[cols, :].reshape(2, 128, C)
                .transpose(1, 0, 2).reshape(128, 2 * C).astype(bf16)),
            "bqk": np.ascontiguousarray(np.stack(
                [bq[cols][0:128], bk[cols][0:128],
                 bq[cols][128:256], bk[cols][128:256]], axis=1)),
            "bv": np.ascontiguousarray(bv[cols].reshape(1, F)),
        })
    return maps


def kernel(q, W_qkv, b_qkv, W_proj, b_proj):
    from concourse.bass_utils import run_bass_kernel_spmd

    q = np.ascontiguousarray(np.asarray(q, dtype=np.float32))
    W_qkv = np.ascontiguousarray(np.asarray(W_qkv, dtype=np.float32))
    b_qkv = np.ascontiguousarray(np.asarray(b_qkv, dtype=np.float32))
    W_proj = np.ascontiguousarray(np.asarray(W_proj, dtype=np.float32))
    b_proj = np.ascontiguousarray(np.asarray(b_proj, dtype=np.float32))

    nc = _get_nc()
    res = run_bass_kernel_spmd(nc, _in_maps(q, W_qkv, b_qkv, W_proj),
                               core_ids=list(range(NCORES)))

    out = np.zeros((B, N, C), dtype=np.float32)
    for core in range(NCORES):
        out[core // HPC] += res.results[core]["y"]
    out += b_proj
    return out


# revision 42
# speedup vs baseline: 1.0523x; 1.0077x over previous
"""Trainium2 Bass kernel for nn_Attention_56470230008033.

Multi-head self-attention (B=2, N=2048, C=1024, H=16 heads, D=64),
k = v = q, full qkv projection + output projection.

Sharding over 8 NeuronCores: data parallel on batch (2) x tensor
parallel on heads (4 head-groups of 4 heads).

Fused streaming design (v2): the scalar engine's exp stream over the
4x2048x2048 attention matrix (~110us of ACT work) is the critical
path, so everything else is scheduled around keeping it dense:
  - x and weights are passed from host in bf16; x is DMA'd with the
    XBAR transpose directly DRAM -> SBUF (no staging, no PE transpose)
  - logits^T per head pair via row-tiled concurrent K=64 matmuls
    (heads at PE rows 0-63 / 64-127, separate PSUM banks)
  - softmax denominators via the ones-column-in-V trick (65-col PV)
  - output projection with K=128 (two heads packed per contraction)
  - per-chunk (512 query rows) pipeline: B -> exp -> PV, with the
    QKV projections for later pairs interleaved into PE slack, and
    y projection + DMA-out streamed per chunk of the second pair
"""

import os
import sys

for _p in ("/opt/trn_rl_repo", "/opt/pypackages"):
    if _p not in sys.path:
        sys.path.append(_p)

import numpy as np

_DEBUG = os.environ.get("KDEBUG") == "1"

B, N, C, H = 2, 2048, 1024, 16
D = C // H            # 64 head dim
NCORES = 8
HPC = 4               # heads per core
F = HPC * D           # 256 features per core
NT = N // 128         # 16 token tiles
CT = C // 128         # 8 contraction tiles
NCH = N // 512        # 4 chunks of 512

_CACHE = {}


def _build():
    from concourse import bacc, bass, mybir, tile, masks

    F32 = mybir.dt.float32
    BF16 = mybir.dt.bfloat16
    AF = mybir.ActivationFunctionType

    nc = bacc.Bacc(
        "TRN2",
        target_bir_lowering=False,
        debug=False,
        enable_asserts=False,
        num_devices=NCORES,
    )
    x_d = nc.dram_tensor("x", [N, C], BF16, kind="ExternalInput")
    # weights arrive partition-packed from the host so each loads in one
    # contiguous near-line-rate DMA. wqk row p = [Q01K01 c=0..7 | Q23K23
    # c=0..7], 256 cols per c-tile ([Q|K] 128 each).
    wqk_d = nc.dram_tensor("wqk", [128, 2 * CT * 256], BF16, kind="ExternalInput")
    wv_d = nc.dram_tensor("wv", [128, CT * F], BF16, kind="ExternalInput")
    wp_d = nc.dram_tensor("wp", [128, 2 * C], BF16, kind="ExternalInput")
    bqk_d = nc.dram_tensor("bqk", [128, 4], F32, kind="ExternalInput")
    bv_d = nc.dram_tensor("bv", [1, F], F32, kind="ExternalInput")
    y_d = nc.dram_tensor("y", [N, C], F32, kind="ExternalOutput")
    if _DEBUG:
        qk_dump = nc.dram_tensor("qk_dump", [4 * 128, N], BF16, kind="ExternalOutput")
        va_dump = nc.dram_tensor("va_dump", [NT * 128, HPC * (D + 1)], BF16,
                                 kind="ExternalOutput")
        ot_dump = nc.dram_tensor("ot_dump", [2 * 128, N], BF16, kind="ExternalOutput")

    with tile.TileContext(nc) as tc:
        from contextlib import ExitStack

        with ExitStack() as ctx:
            const = ctx.enter_context(tc.tile_pool(name="const", bufs=1))
            persist = ctx.enter_context(tc.tile_pool(name="persist", bufs=1))
            ptp = ctx.enter_context(tc.tile_pool(name="ptp", bufs=8))
            ysb = ctx.enter_context(tc.tile_pool(name="ysb", bufs=2))
            snorm = ctx.enter_context(tc.tile_pool(name="snorm", bufs=2))

            ident = const.tile([128, 128], BF16, name="ident", tag="ident")
            masks.make_identity(nc, ident[:])
            junk = const.tile([128, 512], BF16, name="junk", tag="junk")
            nc.vector.memset(junk[:], 0.0)
            ones64 = const.tile([1, D], F32, name="ones64", tag="ones64")
            nc.vector.memset(ones64[:], 1.0)

            # persistent SBUF tensors (all bf16 from host)
            xTall = persist.tile([128, CT * N], BF16, name="xTall", tag="xTall")
            xT3 = xTall.rearrange("p (c n) -> p c n", c=CT)
            wqk3 = persist.tile([128, 2, CT, 256], BF16, name="wqk3", tag="wqk3")
            wv3 = persist.tile([128, CT, F], BF16, name="wv3", tag="wv3")
            wpp3 = persist.tile([128, 2, C], BF16, name="wpp3", tag="wpp3")
            # qkT[0]=Q01 qkT[1]=K01 qkT[2]=Q23 qkT[3]=K23; per pair the
            # even head sits at rows 0-63, odd head at rows 64-127
            qkT = [persist.tile([128, N], BF16, name=f"qkT{f}", tag=f"qkT{f}")
                   for f in range(4)]
            # V with ones column per head: [128, h, 65]
            vaug = [persist.tile([128, HPC, D + 1], BF16, name=f"vaug{t}", tag=f"vaug{t}")
                    for t in range(NT)]
            oTp = [persist.tile([128, N], BF16, name=f"oTp{hp}", tag=f"oTp{hp}")
                   for hp in range(2)]
            bqk_sb = const.tile([128, 4], F32, name="bqk_sb", tag="bqk_sb")
            bvb = const.tile([128, F], F32, name="bvb", tag="bvb")
            bvb3 = bvb.rearrange("p (h d) -> p h d", h=HPC)

            # ---- front-loaded DMA issue, all on the sync (HWDGE) ring.
            # DMA_TRANSPOSE serializes against other DMA traffic, so the
            # weight loads are interleaved between the 4 x-transposes in
            # consumption order rather than on a second ring.
            def xpose(g):
                nc.sync.dma_start(xT3[:, :, g * 512:(g + 1) * 512],
                                  x_d.ap()[g * 512:(g + 1) * 512, :],
                                  transpose=True)

            wqk_dv = wqk_d.ap().rearrange("p (w c f) -> p w c f", w=2, c=CT)
            nc.sync.dma_start(wqk3[:, 0], wqk_dv[:, 0])      # Q01|K01
            nc.sync.dma_start(bqk_sb[:], bqk_d.ap())
            nc.sync.dma_start(wv3[:], wv_d.ap().rearrange("p (c f) -> p c f", c=CT))
            bv1 = const.tile([1, F], F32, name="bv1", tag="bv1")
            nc.sync.dma_start(bv1[:], bv_d.ap())
            nc.gpsimd.partition_broadcast(bvb[:], bv1[:])
            # the DIRECT2D->TRANSPOSE transition pays a ~3us serialization
            # gap, so issue all four transposes back-to-back after the
            # pair-0-critical weights
            xpose(0)
            xpose(1)
            xpose(2)
            xpose(3)
            # pair-1 weights are not needed until ~mid-stream; keep them
            # behind the transposes so chunk 0's K tiles arrive sooner
            nc.sync.dma_start(wqk3[:, 1], wqk_dv[:, 1])      # Q23|K23
            nc.sync.dma_start(wpp3[:], wp_d.ap().rearrange("p (a c) -> p a c", a=2))

            # exp table preload on the scalar engine (one-time ~2.7us)
            scr = const.tile([1, 16], F32, name="scr", tag="scr")
            nc.scalar.activation(scr[:], ident[0:1, 0:16], AF.Exp)

            # ones columns of vaug (never overwritten afterwards)
            for t in range(NT):
                nc.vector.memset(vaug[t][:, :, D:D + 1], 1.0)

            with tc.tile_pool(name="bpp", bufs=2, space=bass.MemorySpace.PSUM) as bpp, \
                 tc.tile_pool(name="cpp", bufs=1, space=bass.MemorySpace.PSUM) as cpp:

                def qk_proj(f, j):
                    # project qkT[f] n-cols j*512:(j+1)*512
                    w, fh = divmod(f, 2)
                    qp = qpp.tile([128, 512], F32, name="qp", tag="qp")
                    for c in range(CT):
                        nc.tensor.matmul(
                            qp[:], wqk3[:, w, c, fh * 128:(fh + 1) * 128],
                            xT3[:, c, j * 512:(j + 1) * 512],
                            start=(c == 0), stop=(c == CT - 1))
                    nc.vector.tensor_scalar_add(
                        qkT[f][:, j * 512:(j + 1) * 512], qp[:], bqk_sb[:, f:f + 1])

                def v_proj(t):
                    vp = vpp.tile([128, F], F32, name="vp", tag="vp")
                    for c in range(CT):
                        nc.tensor.matmul(
                            vp[:], xT3[:, c, t * 128:(t + 1) * 128], wv3[:, c],
                            start=(c == 0), stop=(c == CT - 1))
                    nc.vector.tensor_add(
                        vaug[t][:, :, 0:D],
                        vp.rearrange("p (h d) -> p h d", h=HPC), bvb3)

                def y_sub(c, s):
                    # per-512-col halves so the psum copy of one half
                    # overlaps the projection of the next
                    t = c * 4 + s
                    ys = ysb.tile([128, C], F32, name="ys", tag="ys")
                    for half in range(2):
                        yp = ypp.tile([128, 512], F32, name="yp", tag="yp")
                        for hp in range(2):
                            nc.tensor.matmul(
                                yp[:],
                                oTp[hp][:, t * 128:(t + 1) * 128],
                                wpp3[:, hp, half * 512:(half + 1) * 512],
                                start=(hp == 0), stop=(hp == 1))
                        nc.vector.tensor_copy(ys[:, half * 512:(half + 1) * 512], yp[:])
                    nc.sync.dma_start(y_d.ap()[t * 128:(t + 1) * 128, :], ys[:])

                def run_chunk(p, c, hooks, pre=(), pre_late=()):
                    qt, kt = qkT[2 * p], qkT[2 * p + 1]
                    nb = c * 512
                    cp = cpp.tile([D + 1, 2, 512], F32, name="cp", tag="cp")

                    def bmm(mt):
                        bp = bpp.tile([128, 2, 512], F32, name="bp", tag="bp")
                        nc.tensor.matmul(
                            bp[:, 0], kt[0:D, mt * 128:(mt + 1) * 128],
                            qt[0:D, nb:nb + 512], start=True, stop=True)
                        nc.tensor.matmul(
                            bp[:, 1], kt[D:2 * D, mt * 128:(mt + 1) * 128],
                            qt[D:2 * D, nb:nb + 512], start=True, stop=True)
                        return bp

                    # bmm(0..3) only need K01 m-chunk 0, so they are emitted
                    # before pre_late (v0/v1/K01-mk1) to unblock the first
                    # exps; bmm(4..5) consume the m-chunk projected there
                    for hook in pre:
                        hook()
                    bps = {mt: bmm(mt) for mt in range(4)}
                    for hook in pre_late:
                        hook()
                    bps.update({mt: bmm(mt) for mt in (4, 5)})
                    for mt in range(NT):
                        pt = ptp.tile([128, 2, 512], BF16, name="pt", tag="pt")
                        nc.scalar.activation(
                            pt.rearrange("p a b -> p (a b)"),
                            bps.pop(mt).rearrange("p a b -> p (a b)"),
                            AF.Exp, scale=float(D) ** -0.5)
                        nc.tensor.matmul(cp[:, 0], vaug[mt][:, 2 * p, :], pt[:, 0],
                                         start=(mt == 0), stop=(mt == NT - 1))
                        nc.tensor.matmul(cp[:, 1], vaug[mt][:, 2 * p + 1, :], pt[:, 1],
                                         start=(mt == 0), stop=(mt == NT - 1))
                        for hook in hooks.get(mt, ()):
                            hook()
                        if mt + 6 < NT:
                            bps[mt + 6] = bmm(mt + 6)
                    # normalize: oTp rows = cp[0:D] * (1/cp[D]) per head,
                    # both heads batched per DVE op. DVE lanes are
                    # partition-locked: the den row is first copied to
                    # partition 0 (tensor_copy handles the cross-partition
                    # move; reciprocal would not), and the odd head goes
                    # through a partition-0 scratch + SBUF->SBUF DMA to
                    # reach partitions 64-127.
                    s0 = snorm.tile([1, 2, 512], F32, name="s0", tag="s0")
                    nc.vector.tensor_copy(s0[:], cp[D:D + 1, :])
                    # stage the value rows to SBUF right away so the cp
                    # psum bank frees before the recip/broadcast chain —
                    # the next chunk's first PV no longer waits on it
                    cbig = snorm.tile([D, 2, 512], F32, name="cbig", tag="cbig")
                    nc.vector.tensor_copy(cbig[:], cp[0:D, :])
                    sr = snorm.tile([1, 2 * 512], F32, name="sr", tag="sr")
                    nc.vector.reciprocal_approx_fast(
                        sr[:], s0.rearrange("p a b -> p (a b)"))
                    sb = snorm.tile([D, 2, 512], F32, name="sb", tag="sb")
                    nc.gpsimd.partition_broadcast(
                        sb.rearrange("p a b -> p (a b)"), sr[:])
                    nc.vector.tensor_mul(
                        oTp[p][0:D, nb:nb + 512], cbig[:, 0], sb[:, 0])
                    om = snorm.tile([D, 512], BF16, name="om", tag="om")
                    nc.vector.tensor_mul(om[:], cbig[:, 1], sb[:, 1])
                    nc.sync.dma_start(oTp[p][D:2 * D, nb:nb + 512], om[:])

                # ---- pair 0: projections + chunk stream ----
                with tc.tile_pool(name="qpp", bufs=1, space=bass.MemorySpace.PSUM) as qpp, \
                     tc.tile_pool(name="vpp", bufs=1, space=bass.MemorySpace.PSUM) as vpp:
                    # HAM warm-up fillers (junk, never read): enough to keep
                    # the PE busy until the first x-transpose lands, so the
                    # Q/K projections run at the warm 2.4 GHz clock
                    for _ in range(16):
                        wt = qpp.tile([128, 512], F32, name="wt", tag="qp")
                        nc.tensor.matmul(wt[:], ident[:], junk[:],
                                         start=True, stop=True)
                    qk_proj(0, 0)      # Q01 chunk 0
                    qk_proj(1, 0)      # K01 m-chunk 0

                    # K01 m-chunk k must be emitted before bmm(4k) — with the
                    # 4-deep bmm look-ahead that means hook slot 4k-4 at the
                    # latest (hooks run before the look-ahead bmm).
                    hooks0 = {mt: [(lambda t=mt + 2: v_proj(t))]
                              for mt in range(NT - 2)}
                    for mk, mt in ((2, 1), (3, 5)):
                        hooks0[mt].append(lambda mk=mk: qk_proj(1, mk))
                    hooks0[12].append(lambda: qk_proj(0, 1))
                    run_chunk(0, 0, hooks0,
                              pre_late=(lambda: v_proj(0), lambda: v_proj(1),
                                        lambda: qk_proj(1, 1)))
                    run_chunk(0, 1, {2: [lambda: qk_proj(3, 0)],
                                     6: [lambda: qk_proj(3, 1)],
                                     10: [lambda: qk_proj(0, 2)]})
                    run_chunk(0, 2, {2: [lambda: qk_proj(3, 2)],
                                     6: [lambda: qk_proj(3, 3)],
                                     10: [lambda: qk_proj(0, 3)]})
                    run_chunk(0, 3, {2: [lambda: qk_proj(2, 0)],
                                     5: [lambda: qk_proj(2, 1)],
                                     8: [lambda: qk_proj(2, 2)],
                                     11: [lambda: qk_proj(2, 3)]})

                # ---- pair 1: chunk stream + streamed y projection ----
                with tc.tile_pool(name="ypp", bufs=2, space=bass.MemorySpace.PSUM) as ypp:
                    run_chunk(1, 0, {})
                    run_chunk(1, 1, {mt: [(lambda s=s: y_sub(0, s))]
                                     for s, mt in enumerate((2, 5, 8, 11))})
                    run_chunk(1, 2, {mt: [(lambda s=s: y_sub(1, s))]
                                     for s, mt in enumerate((2, 5, 8, 11))})
                    run_chunk(1, 3, {mt: [(lambda s=s: y_sub(2, s))]
                                     for s, mt in enumerate((2, 5, 8, 11))})
                    for s in range(4):
                        y_sub(3, s)

                if _DEBUG:
                    for f in range(4):
                        nc.sync.dma_start(qk_dump.ap()[f * 128:(f + 1) * 128, :],
                                          qkT[f][:])
                    for t in range(NT):
                        nc.sync.dma_start(va_dump.ap()[t * 128:(t + 1) * 128, :],
                                          vaug[t].rearrange("p h d -> p (h d)"))
                    for hp in range(2):
                        nc.sync.dma_start(ot_dump.ap()[hp * 128:(hp + 1) * 128, :],
                                          oTp[hp][:])

    nc.compile()
    return nc


def _get_nc():
    if "nc" not in _CACHE:
        _CACHE["nc"] = _build()
    return _CACHE["nc"]


def _in_maps(q, W_qkv, b_qkv, W_proj):
    import ml_dtypes

    bf16 = ml_dtypes.bfloat16
    maps = []
    Wq, Wk, Wv = W_qkv[:, :C], W_qkv[:, C:2 * C], W_qkv[:, 2 * C:]
    bq, bk, bv = b_qkv[:C], b_qkv[C:2 * C], b_qkv[2 * C:]
    def pack(w):
        # [1024, X] -> partition-packed [128, 8*X]: row p = [c=0..7 slices]
        return w.reshape(CT, 128, -1).transpose(1, 0, 2).reshape(128, -1)

    for core in range(NCORES):
        b, g = divmod(core, HPC)
        cols = slice(g * F, (g + 1) * F)
        wqg, wkg = Wq[:, cols], Wk[:, cols]
        # per c-tile cols: [Q01|K01] then [Q23|K23]
        w0 = pack(np.concatenate([wqg[:, 0:128], wkg[:, 0:128]], axis=1))
        w1 = pack(np.concatenate([wqg[:, 128:256], wkg[:, 128:256]], axis=1))
        maps.append({
            "x": np.ascontiguousarray(q[b].astype(bf16)),
            "wqk": np.ascontiguousarray(
                np.concatenate([w0, w1], axis=1).astype(bf16)),
            "wv": np.ascontiguousarray(pack(Wv[:, cols]).astype(bf16)),
            "wp": np.ascontiguousarray(
                W_proj[cols, :].reshape(2, 128, C)
                .transpose(1, 0, 2).reshape(128, 2 * C).astype(bf16)),
            "bqk": np.ascontiguousarray(np.stack(
                [bq[cols][0:128], bk[cols][0:128],
                 bq[cols][128:256], bk[cols][128:256]], axis=1)),
            "bv": np.ascontiguousarray(bv[cols].reshape(1, F)),
        })
    return maps


def kernel(q, W_qkv, b_qkv, W_proj, b_proj):
    from concourse.bass_utils import run_bass_kernel_spmd

    q = np.ascontiguousarray(np.asarray(q, dtype=np.float32))
    W_qkv = np.ascontiguousarray(np.asarray(W_qkv, dtype=np.float32))
    b_qkv = np.ascontiguousarray(np.asarray(b_qkv, dtype=np.float32))
    W_proj = np.ascontiguousarray(np.asarray(W_proj, dtype=np.float32))
    b_proj = np.ascontiguousarray(np.asarray(b_proj, dtype=np.float32))

    nc = _get_nc()
    res = run_bass_kernel_spmd(nc, _in_maps(q, W_qkv, b_qkv, W_proj),
                               core_ids=list(range(NCORES)))

    out = np.zeros((B, N, C), dtype=np.float32)
    for core in range(NCORES):
        out[core // HPC] += res.results[core]["y"]
    out += b_proj
    return out
